# revision 17
# baseline (speedup 1.0000x reference)
"""Cross multi-head attention kernel for 8 Trainium2 NeuronCores.

Reference computation (per batch b):
    Q = x @ Wq.T ; K = ctx @ Wk.T ; V = ctx @ Wv.T          (16 heads, depth 64)
    scores = (Q_h @ K_h.T) / 8 ; masked where pad_mask -> -inf
    att = softmax(scores) ; out_h = att @ V_h
    y = concat_h(out_h) @ fc_w.T + fc_b

Sharding: 8 cores = 2 batches x 4 head-groups (4 heads each).  Data parallel
over B, tensor parallel over heads (Wq/Wk/Wv split column-wise, fc row-wise).
Each core computes a full [E, LQ] partial of y^T for its batch; the host sums
the 4 head-group partials per batch and adds the bias.

On-chip layout (per core) is fully transposed ("layout B") so no transposes
are ever needed on-chip:
    x^T [E, LQ], ctx^T [E, LKV]  ->  Q^T [D,LQ], K^T [D,LKV] per head (matmul
    outputs land transposed for free), V natural [LKV, D] (ctx^T is lhsT).
    scores^T [LKV, LQ] = K^T.T @ Q^T       (contraction over D=64)
    att^T = exp(scores^T * 0.125) * keep_mask^T   (exact-zero masking; no
        row-max needed: scores ~ N(0,1), exp never overflows fp32)
    V is augmented with a ones column -> att@V matmul also emits the softmax
        row-sums (row 64 of the [65, LQ] accumulator) for free.
    normalization: recip(rowsum) broadcast over partitions via a K=1 matmul
        outer product, then one fused multiply during PSUM evacuation.
    y^T partial [E, LQ] = fcw_part^T.T @ attn^T   (contraction over 256)

The exp output / mask / att@V run in bf16 (fp32 accumulation in PSUM); all
projections, scores and fc matmuls are fp32.
"""

import os
import sys

import numpy as np

for _p in ("/opt/trn_rl_repo", "/root/.axon_site/_ro/trn_rl_repo"):
    if os.path.isdir(_p) and _p not in sys.path:
        sys.path.insert(0, _p)

import ml_dtypes  # noqa: E402

import concourse.bass as bass  # noqa: E402
import concourse.mybir as mybir  # noqa: E402
import concourse.tile as tile  # noqa: E402
from concourse import bacc  # noqa: E402
from concourse.bass_utils import run_bass_kernel_spmd  # noqa: E402

B, LQ, LKV, E = 2, 1024, 2048, 1024
H_TOTAL, D = 16, 64
NCORES = 8
HGROUPS = 4          # head groups (cores per batch)
HLOCAL = 4           # heads per core
FP = HLOCAL * D      # 256 local head features
P = 128
F32 = mybir.dt.float32
BF16 = mybir.dt.bfloat16
ET = E // P          # 8 contraction tiles for the projections
KT = LKV // P        # 16 key tiles
NQ = LQ // 512       # 2 fp32 matmul free-dim chunks


def build_nc(debug_taps: bool = False) -> bass.Bass:
    nc = bacc.Bacc("TRN2", target_bir_lowering=False)

    xT = nc.dram_tensor("xT", [E, LQ], F32, kind="ExternalInput")
    ctxT = nc.dram_tensor("ctxT", [E, LKV], F32, kind="ExternalInput")
    maskT = nc.dram_tensor("maskT", [LKV, LQ], BF16, kind="ExternalInput")
    wqT = nc.dram_tensor("wqT", [E, FP], F32, kind="ExternalInput")
    wkT = nc.dram_tensor("wkT", [E, FP], F32, kind="ExternalInput")
    wvT = nc.dram_tensor("wvT", [E, FP], F32, kind="ExternalInput")
    fcwT = nc.dram_tensor("fcwT", [FP, E], F32, kind="ExternalInput")
    yT = nc.dram_tensor("yT", [E, LQ], F32, kind="ExternalOutput")
    if debug_taps:
        qt_dbg = nc.dram_tensor("qt_dbg", [P, 2, LQ], F32, kind="ExternalOutput")
        kt_dbg = nc.dram_tensor("kt_dbg", [P, 2, LKV], F32, kind="ExternalOutput")
        va_dbg = nc.dram_tensor("va_dbg", [P, KT, HLOCAL, D + 1], BF16, kind="ExternalOutput")
        at_dbg = nc.dram_tensor("at_dbg", [P, 2, LQ], F32, kind="ExternalOutput")
        rs_dbg = nc.dram_tensor("rs_dbg", [HLOCAL, LQ], F32, kind="ExternalOutput")
        rc_dbg = nc.dram_tensor("rc_dbg", [HLOCAL, LQ], F32, kind="ExternalOutput")
        bc_dbg = nc.dram_tensor("bc_dbg", [HLOCAL, D, LQ], F32, kind="ExternalOutput")
        ex_dbg = nc.dram_tensor("ex_dbg", [2, P, LQ], BF16, kind="ExternalOutput")

    with tile.TileContext(nc) as tc:
        with tc.tile_pool(name="persist", bufs=1) as persist:
            QT = persist.tile([P, 2, LQ], F32)        # [:, pair, :]; head 2p on rows 0:64
            KTt = persist.tile([P, 2, LKV], F32)
            Vaug = persist.tile([P, KT, HLOCAL, D + 1], BF16)
            attnT = persist.tile([P, 2, LQ], F32)
            fcw_s = persist.tile([P, 2, E], F32)
            # zero-padded broadcast operands: row 0 live, rows 1-127 zero so the
            # K=128 outer-product matmul is exact (K<128 matmuls read garbage
            # rows on HW - tile_size rounds up to 32).
            ones64 = persist.tile([P, D], F32)
            rsr_pad = persist.tile([P, LQ], F32)

            nc.vector.memset(ones64[:], 0.0)
            nc.vector.memset(ones64[0:1, :], 1.0)
            nc.vector.memset(rsr_pad[:], 0.0)
            nc.gpsimd.memset(Vaug[:], 1.0)            # ones column survives; V overwrites the rest
            nc.sync.dma_start(fcw_s[:], fcwT.rearrange("(ko pi) e -> pi ko e", pi=P))

            # ---------------- Phase A: Q/K/V projections ----------------
            with (
                tc.tile_pool(name="inp", bufs=1) as inp,
                tc.tile_pool(name="psumA", bufs=2, space="PSUM") as psumA,
            ):
                xT_s = []
                cT_s = []
                wq_s = []
                wk_s = []
                wv_s = []
                for k in range(ET):
                    xt = inp.tile([P, LQ], F32, tag=f"xT{k}")
                    nc.sync.dma_start(xt[:], xT[k * P:(k + 1) * P, :])
                    xT_s.append(xt)
                    ct = inp.tile([P, LKV], F32, tag=f"cT{k}")
                    nc.sync.dma_start(ct[:], ctxT[k * P:(k + 1) * P, :])
                    cT_s.append(ct)
                    wq = inp.tile([P, FP], F32, tag=f"wq{k}")
                    nc.sync.dma_start(wq[:], wqT[k * P:(k + 1) * P, :])
                    wq_s.append(wq)
                    wk = inp.tile([P, FP], F32, tag=f"wk{k}")
                    nc.sync.dma_start(wk[:], wkT[k * P:(k + 1) * P, :])
                    wk_s.append(wk)
                    wv = inp.tile([P, FP], F32, tag=f"wv{k}")
                    nc.sync.dma_start(wv[:], wvT[k * P:(k + 1) * P, :])
                    wv_s.append(wv)

                # Q^T [FP, LQ] in pair-major tiles
                for p in range(2):
                    for n in range(NQ):
                        ps = psumA.tile([P, 512], F32, tag="ps512")
                        for k in range(ET):
                            nc.tensor.matmul(
                                ps[:],
                                wq_s[k][:, p * P:(p + 1) * P],
                                xT_s[k][:, n * 512:(n + 1) * 512],
                                start=(k == 0),
                                stop=(k == ET - 1),
                            )
                        nc.vector.tensor_copy(QT[:, p, n * 512:(n + 1) * 512], ps[:])

                # K^T [FP, LKV]
                for p in range(2):
                    for n in range(LKV // 512):
                        ps = psumA.tile([P, 512], F32, tag="ps512")
                        for k in range(ET):
                            nc.tensor.matmul(
                                ps[:],
                                wk_s[k][:, p * P:(p + 1) * P],
                                cT_s[k][:, n * 512:(n + 1) * 512],
                                start=(k == 0),
                                stop=(k == ET - 1),
                            )
                        nc.vector.tensor_copy(KTt[:, p, n * 512:(n + 1) * 512], ps[:])

                # V natural [LKV, FP] scattered into the ones-augmented tile
                for mv in range(KT):
                    ps = psumA.tile([P, FP], F32, tag="psv")
                    for k in range(ET):
                        nc.tensor.matmul(
                            ps[:],
                            cT_s[k][:, mv * P:(mv + 1) * P],
                            wv_s[k][:],
                            start=(k == 0),
                            stop=(k == ET - 1),
                        )
                    nc.vector.tensor_copy(
                        Vaug[:, mv, :, 0:D],
                        ps.rearrange("p (h d) -> p h d", d=D),
                    )

            if debug_taps:
                nc.sync.dma_start(qt_dbg[:], QT[:])
                nc.sync.dma_start(kt_dbg[:], KTt[:])
                nc.sync.dma_start(va_dbg[:], Vaug[:])

            # ---------------- Phase B: attention ----------------
            with (
                tc.tile_pool(name="maskp", bufs=1) as maskp,
                tc.tile_pool(name="work", bufs=4) as work,
                tc.tile_pool(name="psumB", bufs=1, space="PSUM") as psumB,
            ):
                mT_s = []
                for kt in range(KT):
                    mt = maskp.tile([P, LQ], BF16, tag=f"m{kt}")
                    nc.sync.dma_start(mt[:], maskT[kt * P:(kt + 1) * P, :])
                    mT_s.append(mt)

                for p in range(2):
                    avs = [
                        psumB.tile([D + 1, LQ], F32, tag="avh0", name="av0"),
                        psumB.tile([D + 1, LQ], F32, tag="avh1", name="av1"),
                    ]
                    for kt in range(KT):
                        for h in range(2):
                            base = h * D
                            sc = psumB.tile([P, LQ], F32, tag="sc", bufs=2)
                            for n in range(NQ):
                                nc.tensor.matmul(
                                    sc[:, n * 512:(n + 1) * 512],
                                    KTt[base:base + D, p, kt * P:(kt + 1) * P],
                                    QT[base:base + D, p, n * 512:(n + 1) * 512],
                                    start=True,
                                    stop=True,
                                )
                            ex = work.tile([P, LQ], BF16, tag="ex")
                            nc.scalar.activation(
                                ex[:], sc[:],
                                mybir.ActivationFunctionType.Exp,
                                scale=0.125,
                            )
                            nc.vector.tensor_tensor(
                                ex[:], ex[:], mT_s[kt][:], mybir.AluOpType.mult
                            )
                            if debug_taps and p == 0 and kt == 0:
                                nc.sync.dma_start(ex_dbg[h], ex[:])
                            for n in range(NQ):
                                nc.tensor.matmul(
                                    avs[h][:, n * 512:(n + 1) * 512],
                                    Vaug[:, kt, 2 * p + h, :],
                                    ex[:, n * 512:(n + 1) * 512],
                                    start=(kt == 0),
                                    stop=(kt == KT - 1),
                                )

                    # softmax normalization + evacuation for this pair
                    for h in range(2):
                        av = avs[h]
                        hh = 2 * p + h
                        if debug_taps:
                            rsd = work.tile([1, LQ], F32, tag="rsd", bufs=2)
                            nc.scalar.copy(rsd[:], av[D:D + 1, :])
                            nc.sync.dma_start(rs_dbg[hh:hh + 1, :], rsd[:])
                        # recip via exp(-ln(x)) on ACT: the custom-DVE
                        # reciprocal_approx ops return garbage on this runtime,
                        # and exact DVE reciprocal is 8 cyc/elem.  ln+exp share
                        # one ACT table set; composition error ~1e-6 rel.
                        lnr = work.tile([1, LQ], F32, tag="lnr", bufs=2)
                        nc.scalar.activation(
                            lnr[:], av[D:D + 1, :], mybir.ActivationFunctionType.Ln
                        )
                        nc.scalar.activation(
                            rsr_pad[0:1, :], lnr[:],
                            mybir.ActivationFunctionType.Exp, scale=-1.0,
                        )
                        if debug_taps:
                            nc.sync.dma_start(rc_dbg[hh:hh + 1, :], rsr_pad[0:1, :])
                        bc = psumB.tile([P, LQ], F32, tag="sc", bufs=2)
                        for n in range(NQ):
                            nc.tensor.matmul(
                                bc[0:D, n * 512:(n + 1) * 512],
                                ones64[:],
                                rsr_pad[:, n * 512:(n + 1) * 512],
                                start=True,
                                stop=True,
                            )
                        bcs = work.tile([D, LQ], F32, tag="bcs", bufs=2)
                        nc.scalar.copy(bcs[:], bc[0:D, :])
                        if debug_taps:
                            nc.sync.dma_start(bc_dbg[hh], bcs[:])
                        nc.vector.tensor_tensor(
                            attnT[h * D:(h + 1) * D, p, :],
                            av[0:D, :],
                            bcs[:],
                            mybir.AluOpType.mult,
                        )

            if debug_taps:
                nc.sync.dma_start(at_dbg[:], attnT[:])

            # ---------------- Phase C: output projection ----------------
            with (
                tc.tile_pool(name="psumC", bufs=2, space="PSUM") as psumC,
                tc.tile_pool(name="outp", bufs=3) as outp,
            ):
                for m in range(ET):
                    ps = psumC.tile([P, LQ], F32, tag="fc")
                    for n in range(NQ):
                        for kf in range(2):
                            nc.tensor.matmul(
                                ps[:, n * 512:(n + 1) * 512],
                                fcw_s[:, kf, m * P:(m + 1) * P],
                                attnT[:, kf, n * 512:(n + 1) * 512],
                                start=(kf == 0),
                                stop=(kf == 1),
                            )
                    ob = outp.tile([P, LQ], F32, tag="ob")
                    nc.vector.tensor_copy(ob[:], ps[:])
                    nc.sync.dma_start(yT[m * P:(m + 1) * P, :], ob[:])

    nc.compile()
    return nc


_NC_CACHE: dict = {}


def _get_nc() -> bass.Bass:
    if "nc" not in _NC_CACHE:
        _NC_CACHE["nc"] = build_nc()
    return _NC_CACHE["nc"]


def make_in_maps(x, context, pad_mask, Wq, Wk, Wv, fc_w):
    x = np.asarray(x, dtype=np.float32)
    context = np.asarray(context, dtype=np.float32)
    pad_mask = np.asarray(pad_mask).astype(bool)
    Wq = np.asarray(Wq, dtype=np.float32)
    Wk = np.asarray(Wk, dtype=np.float32)
    Wv = np.asarray(Wv, dtype=np.float32)
    fc_w = np.asarray(fc_w, dtype=np.float32)

    xT = np.ascontiguousarray(x.transpose(0, 2, 1))                 # [B, E, LQ]
    cT = np.ascontiguousarray(context.transpose(0, 2, 1))           # [B, E, LKV]
    keepT = np.ascontiguousarray(
        (~pad_mask).transpose(0, 2, 1)
    ).astype(ml_dtypes.bfloat16)                                    # [B, LKV, LQ]

    in_maps = []
    for c in range(NCORES):
        b, hg = divmod(c, HGROUPS)
        fsl = slice(hg * FP, (hg + 1) * FP)
        in_maps.append(
            {
                "xT": xT[b],
                "ctxT": cT[b],
                "maskT": keepT[b],
                "wqT": np.ascontiguousarray(Wq[fsl, :].T),
                "wkT": np.ascontiguousarray(Wk[fsl, :].T),
                "wvT": np.ascontiguousarray(Wv[fsl, :].T),
                "fcwT": np.ascontiguousarray(fc_w[:, fsl].T),
            }
        )
    return in_maps


def _combine(outs, fc_b):
    fc_b = np.asarray(fc_b, dtype=np.float32)
    y = np.empty((B, LQ, E), dtype=np.float32)
    for b in range(B):
        acc = outs[HGROUPS * b]
        for g in range(1, HGROUPS):
            acc = acc + outs[HGROUPS * b + g]
        y[b] = acc.T + fc_b
    return y


def run_traced(x, context, pad_mask, Wq, Wk, Wv, fc_w, fc_b, trace=False):
    nc = _get_nc()
    in_maps = make_in_maps(x, context, pad_mask, Wq, Wk, Wv, fc_w)
    res = run_bass_kernel_spmd(nc, in_maps, list(range(NCORES)), trace=trace)
    outs = [r["yT"] for r in res.results]
    return _combine(outs, fc_b), res


def kernel(x, context, pad_mask, Wq, Wk, Wv, fc_w, fc_b):
    y, _ = run_traced(x, context, pad_mask, Wq, Wk, Wv, fc_w, fc_b, trace=False)
    return y


# revision 19
# speedup vs baseline: 1.5680x; 1.5680x over previous
"""Cross multi-head attention kernel for 8 Trainium2 NeuronCores.

Reference computation (per batch b):
    Q = x @ Wq.T ; K = ctx @ Wk.T ; V = ctx @ Wv.T          (16 heads, depth 64)
    scores = (Q_h @ K_h.T) / 8 ; masked where pad_mask -> -inf
    att = softmax(scores) ; out_h = att @ V_h
    y = concat_h(out_h) @ fc_w.T + fc_b

Sharding: 8 cores = 2 batches x 4 head-groups (4 heads each).  Data parallel
over B, tensor parallel over heads (Wq/Wk/Wv split column-wise, fc row-wise).
Each core computes a full [E, LQ] partial of y^T for its batch; the host sums
the 4 head-group partials per batch and adds the bias.

On-chip layout (per core) is fully transposed ("layout B") so no transposes
are ever needed on-chip:
    x^T [E, LQ], ctx^T [E, LKV]  ->  Q^T [D,LQ], K^T [D,LKV] per head (matmul
    outputs land transposed for free), V natural [LKV, D] (ctx^T is lhsT).
    scores^T [LKV, LQ] = K^T.T @ Q^T       (contraction over D=64)
    att^T = exp(scores^T * 0.125) * keep_mask^T   (exact-zero masking; no
        row-max needed: scores ~ N(0,1), exp never overflows fp32)
    V is augmented with a ones column -> att@V matmul also emits the softmax
        row-sums (row 64 of the [65, LQ] accumulator) for free.
    normalization: recip(rowsum) broadcast over partitions via a K=1 matmul
        outer product, then one fused multiply during PSUM evacuation.
    y^T partial [E, LQ] = fcw_part^T.T @ attn^T   (contraction over 256)

The exp output / mask / att@V run in bf16 (fp32 accumulation in PSUM); all
projections, scores and fc matmuls are fp32.
"""

import os
import sys

import numpy as np

for _p in ("/opt/trn_rl_repo", "/root/.axon_site/_ro/trn_rl_repo"):
    if os.path.isdir(_p) and _p not in sys.path:
        sys.path.insert(0, _p)

import ml_dtypes  # noqa: E402

import concourse.bass as bass  # noqa: E402
import concourse.mybir as mybir  # noqa: E402
import concourse.tile as tile  # noqa: E402
from concourse import bacc  # noqa: E402
from concourse.bass_utils import run_bass_kernel_spmd  # noqa: E402

B, LQ, LKV, E = 2, 1024, 2048, 1024
H_TOTAL, D = 16, 64
NCORES = 8
HGROUPS = 4          # head groups (cores per batch)
HLOCAL = 4           # heads per core
FP = HLOCAL * D      # 256 local head features
P = 128
F32 = mybir.dt.float32
F32R = mybir.dt.float32r
BF16 = mybir.dt.bfloat16
ET = E // P          # 8 contraction tiles for the projections
KT = LKV // P        # 16 key tiles
NQ = LQ // 512       # 2 fp32 matmul free-dim chunks


def build_nc(debug_taps: bool = False) -> bass.Bass:
    nc = bacc.Bacc("TRN2", target_bir_lowering=False)

    xT = nc.dram_tensor("xT", [E, LQ], F32R, kind="ExternalInput")
    ctxT = nc.dram_tensor("ctxT", [E, LKV], F32R, kind="ExternalInput")
    maskT = nc.dram_tensor("maskT", [LKV, LQ], BF16, kind="ExternalInput")
    wqT = nc.dram_tensor("wqT", [E, FP], F32R, kind="ExternalInput")
    wkT = nc.dram_tensor("wkT", [E, FP], F32R, kind="ExternalInput")
    wvT = nc.dram_tensor("wvT", [E, FP], F32R, kind="ExternalInput")
    fcwT = nc.dram_tensor("fcwT", [FP, E], BF16, kind="ExternalInput")
    yT = nc.dram_tensor("yT", [E, LQ], F32, kind="ExternalOutput")
    if debug_taps:
        qt_dbg = nc.dram_tensor("qt_dbg", [P, 2, LQ], BF16, kind="ExternalOutput")
        kt_dbg = nc.dram_tensor("kt_dbg", [P, 2, LKV], BF16, kind="ExternalOutput")
        va_dbg = nc.dram_tensor("va_dbg", [P, KT, HLOCAL, D + 1], BF16, kind="ExternalOutput")
        at_dbg = nc.dram_tensor("at_dbg", [P, 2, LQ], BF16, kind="ExternalOutput")
        rs_dbg = nc.dram_tensor("rs_dbg", [HLOCAL, LQ], F32, kind="ExternalOutput")
        rc_dbg = nc.dram_tensor("rc_dbg", [HLOCAL, LQ], F32, kind="ExternalOutput")
        bc_dbg = nc.dram_tensor("bc_dbg", [HLOCAL, D, LQ], F32, kind="ExternalOutput")
        ex_dbg = nc.dram_tensor("ex_dbg", [2, P, LQ], BF16, kind="ExternalOutput")

    with tile.TileContext(nc) as tc:
        with tc.tile_pool(name="persist", bufs=1) as persist:
            QT = persist.tile([P, 2, LQ], BF16)        # [:, pair, :]; head 2p on rows 0:64
            KTt = persist.tile([P, 2, LKV], BF16)
            Vaug = persist.tile([P, KT, HLOCAL, D + 1], BF16)
            attnT = persist.tile([P, 2, LQ], BF16)
            fcw_s = persist.tile([P, 2, E], BF16)
            # zero-padded broadcast operands: row 0 live, rows 1-127 zero so the
            # K=128 outer-product matmul is exact (K<128 matmuls read garbage
            # rows on HW - tile_size rounds up to 32).
            ones64 = persist.tile([P, D], F32)
            rsr_pad = persist.tile([P, LQ], F32)

            nc.vector.memset(ones64[:], 0.0)
            nc.vector.memset(ones64[0:1, :], 1.0)
            nc.vector.memset(rsr_pad[:], 0.0)
            nc.gpsimd.memset(Vaug[:], 1.0)            # ones column survives; V overwrites the rest
            nc.sync.dma_start(fcw_s[:], fcwT.rearrange("(ko pi) e -> pi ko e", pi=P))

            # ---------------- Phase A: Q/K/V projections ----------------
            with (
                tc.tile_pool(name="inp", bufs=1) as inp,
                tc.tile_pool(name="psumA", bufs=2, space="PSUM") as psumA,
            ):
                xT_s = []
                cT_s = []
                wq_s = []
                wk_s = []
                wv_s = []
                for k in range(ET):
                    xt = inp.tile([P, LQ], F32R, tag=f"xT{k}")
                    nc.sync.dma_start(xt[:], xT[k * P:(k + 1) * P, :])
                    xT_s.append(xt)
                    ct = inp.tile([P, LKV], F32R, tag=f"cT{k}")
                    nc.sync.dma_start(ct[:], ctxT[k * P:(k + 1) * P, :])
                    cT_s.append(ct)
                    wq = inp.tile([P, FP], F32R, tag=f"wq{k}")
                    nc.sync.dma_start(wq[:], wqT[k * P:(k + 1) * P, :])
                    wq_s.append(wq)
                    wk = inp.tile([P, FP], F32R, tag=f"wk{k}")
                    nc.sync.dma_start(wk[:], wkT[k * P:(k + 1) * P, :])
                    wk_s.append(wk)
                    wv = inp.tile([P, FP], F32R, tag=f"wv{k}")
                    nc.sync.dma_start(wv[:], wvT[k * P:(k + 1) * P, :])
                    wv_s.append(wv)

                # Q^T [FP, LQ] in pair-major tiles
                for p in range(2):
                    for n in range(NQ):
                        ps = psumA.tile([P, 512], F32, tag="ps512")
                        for k in range(ET):
                            nc.tensor.matmul(
                                ps[:],
                                wq_s[k][:, p * P:(p + 1) * P],
                                xT_s[k][:, n * 512:(n + 1) * 512],
                                start=(k == 0),
                                stop=(k == ET - 1),
                            )
                        nc.vector.tensor_copy(QT[:, p, n * 512:(n + 1) * 512], ps[:])

                # K^T [FP, LKV]
                for p in range(2):
                    for n in range(LKV // 512):
                        ps = psumA.tile([P, 512], F32, tag="ps512")
                        for k in range(ET):
                            nc.tensor.matmul(
                                ps[:],
                                wk_s[k][:, p * P:(p + 1) * P],
                                cT_s[k][:, n * 512:(n + 1) * 512],
                                start=(k == 0),
                                stop=(k == ET - 1),
                            )
                        nc.vector.tensor_copy(KTt[:, p, n * 512:(n + 1) * 512], ps[:])

                # V natural [LKV, FP] scattered into the ones-augmented tile
                for mv in range(KT):
                    ps = psumA.tile([P, FP], F32, tag="psv")
                    for k in range(ET):
                        nc.tensor.matmul(
                            ps[:],
                            cT_s[k][:, mv * P:(mv + 1) * P],
                            wv_s[k][:],
                            start=(k == 0),
                            stop=(k == ET - 1),
                        )
                    nc.vector.tensor_copy(
                        Vaug[:, mv, :, 0:D],
                        ps.rearrange("p (h d) -> p h d", d=D),
                    )

            if debug_taps:
                nc.sync.dma_start(qt_dbg[:], QT[:])
                nc.sync.dma_start(kt_dbg[:], KTt[:])
                nc.sync.dma_start(va_dbg[:], Vaug[:])

            # ---------------- Phase B: attention ----------------
            with (
                tc.tile_pool(name="maskp", bufs=1) as maskp,
                tc.tile_pool(name="work", bufs=4) as work,
                tc.tile_pool(name="psumB", bufs=1, space="PSUM") as psumB,
            ):
                mT_s = []
                for kt in range(KT):
                    mt = maskp.tile([P, LQ], BF16, tag=f"m{kt}")
                    nc.sync.dma_start(mt[:], maskT[kt * P:(kt + 1) * P, :])
                    mT_s.append(mt)

                for p in range(2):
                    avs = [
                        psumB.tile([D + 1, LQ], F32, tag="avh0", name="av0"),
                        psumB.tile([D + 1, LQ], F32, tag="avh1", name="av1"),
                    ]
                    for kt in range(KT):
                        for h in range(2):
                            base = h * D
                            sc = psumB.tile([P, LQ], F32, tag="sc", bufs=2)
                            for n in range(NQ):
                                nc.tensor.matmul(
                                    sc[:, n * 512:(n + 1) * 512],
                                    KTt[base:base + D, p, kt * P:(kt + 1) * P],
                                    QT[base:base + D, p, n * 512:(n + 1) * 512],
                                    start=True,
                                    stop=True,
                                )
                            ex = work.tile([P, LQ], BF16, tag="ex")
                            nc.scalar.activation(
                                ex[:], sc[:],
                                mybir.ActivationFunctionType.Exp,
                                scale=0.125,
                            )
                            nc.vector.tensor_tensor(
                                ex[:], ex[:], mT_s[kt][:], mybir.AluOpType.mult
                            )
                            if debug_taps and p == 0 and kt == 0:
                                nc.sync.dma_start(ex_dbg[h], ex[:])
                            for n in range(NQ):
                                nc.tensor.matmul(
                                    avs[h][:, n * 512:(n + 1) * 512],
                                    Vaug[:, kt, 2 * p + h, :],
                                    ex[:, n * 512:(n + 1) * 512],
                                    start=(kt == 0),
                                    stop=(kt == KT - 1),
                                )

                    # softmax normalization + evacuation for this pair
                    for h in range(2):
                        av = avs[h]
                        hh = 2 * p + h
                        if debug_taps:
                            rsd = work.tile([1, LQ], F32, tag="rsd", bufs=2)
                            nc.scalar.copy(rsd[:], av[D:D + 1, :])
                            nc.sync.dma_start(rs_dbg[hh:hh + 1, :], rsd[:])
                        # recip via exp(-ln(x)) on ACT: the custom-DVE
                        # reciprocal_approx ops return garbage on this runtime,
                        # and exact DVE reciprocal is 8 cyc/elem.  ln+exp share
                        # one ACT table set; composition error ~1e-6 rel.
                        lnr = work.tile([1, LQ], F32, tag="lnr", bufs=2)
                        nc.scalar.activation(
                            lnr[:], av[D:D + 1, :], mybir.ActivationFunctionType.Ln
                        )
                        nc.scalar.activation(
                            rsr_pad[0:1, :], lnr[:],
                            mybir.ActivationFunctionType.Exp, scale=-1.0,
                        )
                        if debug_taps:
                            nc.sync.dma_start(rc_dbg[hh:hh + 1, :], rsr_pad[0:1, :])
                        bc = psumB.tile([P, LQ], F32, tag="sc", bufs=2)
                        for n in range(NQ):
                            nc.tensor.matmul(
                                bc[0:D, n * 512:(n + 1) * 512],
                                ones64[:],
                                rsr_pad[:, n * 512:(n + 1) * 512],
                                start=True,
                                stop=True,
                            )
                        bcs = work.tile([D, LQ], F32, tag="bcs", bufs=2)
                        nc.scalar.copy(bcs[:], bc[0:D, :])
                        if debug_taps:
                            nc.sync.dma_start(bc_dbg[hh], bcs[:])
                        nc.vector.tensor_tensor(
                            attnT[h * D:(h + 1) * D, p, :],
                            av[0:D, :],
                            bcs[:],
                            mybir.AluOpType.mult,
                        )

            if debug_taps:
                nc.sync.dma_start(at_dbg[:], attnT[:])

            # ---------------- Phase C: output projection ----------------
            with (
                tc.tile_pool(name="psumC", bufs=2, space="PSUM") as psumC,
                tc.tile_pool(name="outp", bufs=3) as outp,
            ):
                for m in range(ET):
                    ps = psumC.tile([P, LQ], F32, tag="fc")
                    for n in range(NQ):
                        for kf in range(2):
                            nc.tensor.matmul(
                                ps[:, n * 512:(n + 1) * 512],
                                fcw_s[:, kf, m * P:(m + 1) * P],
                                attnT[:, kf, n * 512:(n + 1) * 512],
                                start=(kf == 0),
                                stop=(kf == 1),
                            )
                    ob = outp.tile([P, LQ], F32, tag="ob")
                    nc.vector.tensor_copy(ob[:], ps[:])
                    nc.sync.dma_start(yT[m * P:(m + 1) * P, :], ob[:])

    nc.compile()
    return nc


_NC_CACHE: dict = {}


def _get_nc() -> bass.Bass:
    if "nc" not in _NC_CACHE:
        _NC_CACHE["nc"] = build_nc()
    return _NC_CACHE["nc"]


def make_in_maps(x, context, pad_mask, Wq, Wk, Wv, fc_w):
    x = np.asarray(x, dtype=np.float32)
    context = np.asarray(context, dtype=np.float32)
    pad_mask = np.asarray(pad_mask).astype(bool)
    Wq = np.asarray(Wq, dtype=np.float32)
    Wk = np.asarray(Wk, dtype=np.float32)
    Wv = np.asarray(Wv, dtype=np.float32)
    fc_w = np.asarray(fc_w, dtype=np.float32)

    xT = np.ascontiguousarray(x.transpose(0, 2, 1))                 # [B, E, LQ]
    cT = np.ascontiguousarray(context.transpose(0, 2, 1))           # [B, E, LKV]
    keepT = np.ascontiguousarray(
        (~pad_mask).transpose(0, 2, 1)
    ).astype(ml_dtypes.bfloat16)                                    # [B, LKV, LQ]

    in_maps = []
    for c in range(NCORES):
        b, hg = divmod(c, HGROUPS)
        fsl = slice(hg * FP, (hg + 1) * FP)
        in_maps.append(
            {
                "xT": xT[b],
                "ctxT": cT[b],
                "maskT": keepT[b],
                "wqT": np.ascontiguousarray(Wq[fsl, :].T),
                "wkT": np.ascontiguousarray(Wk[fsl, :].T),
                "wvT": np.ascontiguousarray(Wv[fsl, :].T),
                "fcwT": np.ascontiguousarray(fc_w[:, fsl].T).astype(ml_dtypes.bfloat16),
            }
        )
    return in_maps


def _combine(outs, fc_b):
    fc_b = np.asarray(fc_b, dtype=np.float32)
    y = np.empty((B, LQ, E), dtype=np.float32)
    for b in range(B):
        acc = outs[HGROUPS * b]
        for g in range(1, HGROUPS):
            acc = acc + outs[HGROUPS * b + g]
        y[b] = acc.T + fc_b
    return y


def run_traced(x, context, pad_mask, Wq, Wk, Wv, fc_w, fc_b, trace=False):
    nc = _get_nc()
    in_maps = make_in_maps(x, context, pad_mask, Wq, Wk, Wv, fc_w)
    res = run_bass_kernel_spmd(nc, in_maps, list(range(NCORES)), trace=trace)
    outs = [r["yT"] for r in res.results]
    return _combine(outs, fc_b), res


def kernel(x, context, pad_mask, Wq, Wk, Wv, fc_w, fc_b):
    y, _ = run_traced(x, context, pad_mask, Wq, Wk, Wv, fc_w, fc_b, trace=False)
    return y


# revision 20
# speedup vs baseline: 1.5885x; 1.0131x over previous
"""Cross multi-head attention kernel for 8 Trainium2 NeuronCores.

Reference computation (per batch b):
    Q = x @ Wq.T ; K = ctx @ Wk.T ; V = ctx @ Wv.T          (16 heads, depth 64)
    scores = (Q_h @ K_h.T) / 8 ; masked where pad_mask -> -inf
    att = softmax(scores) ; out_h = att @ V_h
    y = concat_h(out_h) @ fc_w.T + fc_b

Sharding: 8 cores = 2 batches x 4 head-groups (4 heads each).  Data parallel
over B, tensor parallel over heads (Wq/Wk/Wv split column-wise, fc row-wise).
Each core computes a full [E, LQ] partial of y^T for its batch; the host sums
the 4 head-group partials per batch and adds the bias.

On-chip layout (per core) is fully transposed ("layout B") so no transposes
are ever needed on-chip:
    x^T [E, LQ], ctx^T [E, LKV]  ->  Q^T [D,LQ], K^T [D,LKV] per head (matmul
    outputs land transposed for free), V natural [LKV, D] (ctx^T is lhsT).
    scores^T [LKV, LQ] = K^T.T @ Q^T       (contraction over D=64)
    att^T = exp(scores^T * 0.125) * keep_mask^T   (exact-zero masking; no
        row-max needed: scores ~ N(0,1), exp never overflows fp32)
    V is augmented with a ones column -> att@V matmul also emits the softmax
        row-sums (row 64 of the [65, LQ] accumulator) for free.
    normalization: recip(rowsum) broadcast over partitions via a K=1 matmul
        outer product, then one fused multiply during PSUM evacuation.
    y^T partial [E, LQ] = fcw_part^T.T @ attn^T   (contraction over 256)

The exp output / mask / att@V run in bf16 (fp32 accumulation in PSUM); all
projections, scores and fc matmuls are fp32.
"""

import os
import sys

import numpy as np

for _p in ("/opt/trn_rl_repo", "/root/.axon_site/_ro/trn_rl_repo"):
    if os.path.isdir(_p) and _p not in sys.path:
        sys.path.insert(0, _p)

import ml_dtypes  # noqa: E402

import concourse.bass as bass  # noqa: E402
import concourse.mybir as mybir  # noqa: E402
import concourse.tile as tile  # noqa: E402
from concourse import bacc  # noqa: E402
from concourse.bass_utils import run_bass_kernel_spmd  # noqa: E402

B, LQ, LKV, E = 2, 1024, 2048, 1024
H_TOTAL, D = 16, 64
NCORES = 8
HGROUPS = 4          # head groups (cores per batch)
HLOCAL = 4           # heads per core
FP = HLOCAL * D      # 256 local head features
P = 128
F32 = mybir.dt.float32
F32R = mybir.dt.float32r
BF16 = mybir.dt.bfloat16
ET = E // P          # 8 contraction tiles for the projections
KT = LKV // P        # 16 key tiles
NQ = LQ // 512       # 2 fp32 matmul free-dim chunks


def build_nc(debug_taps: bool = False) -> bass.Bass:
    nc = bacc.Bacc("TRN2", target_bir_lowering=False)

    xT = nc.dram_tensor("xT", [E, LQ], F32R, kind="ExternalInput")
    ctxT = nc.dram_tensor("ctxT", [E, LKV], F32R, kind="ExternalInput")
    maskT = nc.dram_tensor("maskT", [LKV, LQ], BF16, kind="ExternalInput")
    wqT = nc.dram_tensor("wqT", [E, FP], F32R, kind="ExternalInput")
    wkT = nc.dram_tensor("wkT", [E, FP], F32R, kind="ExternalInput")
    wvT = nc.dram_tensor("wvT", [E, FP], F32R, kind="ExternalInput")
    fcwT = nc.dram_tensor("fcwT", [FP, E], BF16, kind="ExternalInput")
    yT = nc.dram_tensor("yT", [E, LQ], F32, kind="ExternalOutput")
    if debug_taps:
        qt_dbg = nc.dram_tensor("qt_dbg", [P, 2, LQ], BF16, kind="ExternalOutput")
        kt_dbg = nc.dram_tensor("kt_dbg", [P, 2, LKV], BF16, kind="ExternalOutput")
        va_dbg = nc.dram_tensor("va_dbg", [P, KT, HLOCAL, D + 1], BF16, kind="ExternalOutput")
        at_dbg = nc.dram_tensor("at_dbg", [P, 2, LQ], BF16, kind="ExternalOutput")
        rs_dbg = nc.dram_tensor("rs_dbg", [HLOCAL, LQ], F32, kind="ExternalOutput")
        rc_dbg = nc.dram_tensor("rc_dbg", [HLOCAL, LQ], F32, kind="ExternalOutput")
        bc_dbg = nc.dram_tensor("bc_dbg", [HLOCAL, D, LQ], F32, kind="ExternalOutput")
        ex_dbg = nc.dram_tensor("ex_dbg", [2, P, LQ], BF16, kind="ExternalOutput")

    with tile.TileContext(nc) as tc:
        with tc.tile_pool(name="persist", bufs=1) as persist:
            QT = persist.tile([P, 2, LQ], BF16)        # [:, pair, :]; head 2p on rows 0:64
            KTt = persist.tile([P, 2, LKV], BF16)
            Vaug = persist.tile([P, KT, HLOCAL, D + 1], BF16)
            attnT = persist.tile([P, 2, LQ], BF16)
            fcw_s = persist.tile([P, 2, E], BF16)
            # zero-padded broadcast operands: row 0 live, rows 1-127 zero so the
            # K=128 outer-product matmul is exact (K<128 matmuls read garbage
            # rows on HW - tile_size rounds up to 32).
            ones64 = persist.tile([P, D], F32)
            rsr_pad = persist.tile([P, LQ], F32)

            nc.vector.memset(ones64[:], 0.0)
            nc.vector.memset(ones64[0:1, :], 1.0)
            nc.vector.memset(rsr_pad[:], 0.0)
            nc.gpsimd.memset(Vaug[:], 1.0)            # ones column survives; V overwrites the rest
            nc.sync.dma_start(fcw_s[:], fcwT.rearrange("(ko pi) e -> pi ko e", pi=P))

            # ---------------- Phase A: Q/K/V projections ----------------
            with (
                tc.tile_pool(name="inp", bufs=1) as inp,
                tc.tile_pool(name="psumA", bufs=2, space="PSUM") as psumA,
            ):
                xT_s = []
                cT_s = []
                wq_s = []
                wk_s = []
                wv_s = []
                for k in range(ET):
                    xt = inp.tile([P, LQ], F32R, tag=f"xT{k}")
                    nc.sync.dma_start(xt[:], xT[k * P:(k + 1) * P, :])
                    xT_s.append(xt)
                    ct = inp.tile([P, LKV], F32R, tag=f"cT{k}")
                    nc.sync.dma_start(ct[:], ctxT[k * P:(k + 1) * P, :])
                    cT_s.append(ct)
                    wq = inp.tile([P, FP], F32R, tag=f"wq{k}")
                    nc.sync.dma_start(wq[:], wqT[k * P:(k + 1) * P, :])
                    wq_s.append(wq)
                    wk = inp.tile([P, FP], F32R, tag=f"wk{k}")
                    nc.sync.dma_start(wk[:], wkT[k * P:(k + 1) * P, :])
                    wk_s.append(wk)
                    wv = inp.tile([P, FP], F32R, tag=f"wv{k}")
                    nc.sync.dma_start(wv[:], wvT[k * P:(k + 1) * P, :])
                    wv_s.append(wv)

                # Q^T [FP, LQ] in pair-major tiles
                for p in range(2):
                    for n in range(NQ):
                        ps = psumA.tile([P, 512], F32, tag="ps512", bufs=3)
                        for k in range(ET):
                            nc.tensor.matmul(
                                ps[:],
                                wq_s[k][:, p * P:(p + 1) * P],
                                xT_s[k][:, n * 512:(n + 1) * 512],
                                start=(k == 0),
                                stop=(k == ET - 1),
                            )
                        nc.vector.tensor_copy(QT[:, p, n * 512:(n + 1) * 512], ps[:])

                # K^T [FP, LKV]
                for p in range(2):
                    for n in range(LKV // 512):
                        ps = psumA.tile([P, 512], F32, tag="ps512", bufs=3)
                        for k in range(ET):
                            nc.tensor.matmul(
                                ps[:],
                                wk_s[k][:, p * P:(p + 1) * P],
                                cT_s[k][:, n * 512:(n + 1) * 512],
                                start=(k == 0),
                                stop=(k == ET - 1),
                            )
                        nc.vector.tensor_copy(KTt[:, p, n * 512:(n + 1) * 512], ps[:])

                # V natural [LKV, FP] scattered into the ones-augmented tile
                for mv in range(KT):
                    ps = psumA.tile([P, FP], F32, tag="psv", bufs=3)
                    for k in range(ET):
                        nc.tensor.matmul(
                            ps[:],
                            cT_s[k][:, mv * P:(mv + 1) * P],
                            wv_s[k][:],
                            start=(k == 0),
                            stop=(k == ET - 1),
                        )
                    nc.vector.tensor_copy(
                        Vaug[:, mv, :, 0:D],
                        ps.rearrange("p (h d) -> p h d", d=D),
                    )

            if debug_taps:
                nc.sync.dma_start(qt_dbg[:], QT[:])
                nc.sync.dma_start(kt_dbg[:], KTt[:])
                nc.sync.dma_start(va_dbg[:], Vaug[:])

            # ---------------- Phase B: attention ----------------
            with (
                tc.tile_pool(name="maskp", bufs=1) as maskp,
                tc.tile_pool(name="work", bufs=4) as work,
                tc.tile_pool(name="psumB", bufs=1, space="PSUM") as psumB,
            ):
                mT_s = []
                for kt in range(KT):
                    mt = maskp.tile([P, LQ], BF16, tag=f"m{kt}")
                    nc.sync.dma_start(mt[:], maskT[kt * P:(kt + 1) * P, :])
                    mT_s.append(mt)

                for p in range(2):
                    avs = [
                        psumB.tile([D + 1, LQ], F32, tag="avh0", name="av0"),
                        psumB.tile([D + 1, LQ], F32, tag="avh1", name="av1"),
                    ]
                    for kt in range(KT):
                        for h in range(2):
                            base = h * D
                            sc = psumB.tile([P, LQ], F32, tag="sc", bufs=2)
                            for n in range(NQ):
                                nc.tensor.matmul(
                                    sc[:, n * 512:(n + 1) * 512],
                                    KTt[base:base + D, p, kt * P:(kt + 1) * P],
                                    QT[base:base + D, p, n * 512:(n + 1) * 512],
                                    start=True,
                                    stop=True,
                                )
                            ex = work.tile([P, LQ], BF16, tag="ex")
                            nc.scalar.activation(
                                ex[:], sc[:],
                                mybir.ActivationFunctionType.Exp,
                                scale=0.125,
                            )
                            nc.vector.tensor_tensor(
                                ex[:], ex[:], mT_s[kt][:], mybir.AluOpType.mult
                            )
                            if debug_taps and p == 0 and kt == 0:
                                nc.sync.dma_start(ex_dbg[h], ex[:])
                            for n in range(NQ):
                                nc.tensor.matmul(
                                    avs[h][:, n * 512:(n + 1) * 512],
                                    Vaug[:, kt, 2 * p + h, :],
                                    ex[:, n * 512:(n + 1) * 512],
                                    start=(kt == 0),
                                    stop=(kt == KT - 1),
                                )

                    # softmax normalization + evacuation for this pair
                    for h in range(2):
                        av = avs[h]
                        hh = 2 * p + h
                        if debug_taps:
                            rsd = work.tile([1, LQ], F32, tag="rsd", bufs=2)
                            nc.scalar.copy(rsd[:], av[D:D + 1, :])
                            nc.sync.dma_start(rs_dbg[hh:hh + 1, :], rsd[:])
                        # recip via exp(-ln(x)) on ACT: the custom-DVE
                        # reciprocal_approx ops return garbage on this runtime,
                        # and exact DVE reciprocal is 8 cyc/elem.  ln+exp share
                        # one ACT table set; composition error ~1e-6 rel.
                        lnr = work.tile([1, LQ], F32, tag="lnr", bufs=2)
                        nc.scalar.activation(
                            lnr[:], av[D:D + 1, :], mybir.ActivationFunctionType.Ln
                        )
                        nc.scalar.activation(
                            rsr_pad[0:1, :], lnr[:],
                            mybir.ActivationFunctionType.Exp, scale=-1.0,
                        )
                        if debug_taps:
                            nc.sync.dma_start(rc_dbg[hh:hh + 1, :], rsr_pad[0:1, :])
                        bc = psumB.tile([P, LQ], F32, tag="sc", bufs=2)
                        for n in range(NQ):
                            nc.tensor.matmul(
                                bc[0:D, n * 512:(n + 1) * 512],
                                ones64[:],
                                rsr_pad[:, n * 512:(n + 1) * 512],
                                start=True,
                                stop=True,
                            )
                        bcs = work.tile([D, LQ], F32, tag="bcs", bufs=2)
                        nc.scalar.copy(bcs[:], bc[0:D, :])
                        if debug_taps:
                            nc.sync.dma_start(bc_dbg[hh], bcs[:])
                        nc.vector.tensor_tensor(
                            attnT[h * D:(h + 1) * D, p, :],
                            av[0:D, :],
                            bcs[:],
                            mybir.AluOpType.mult,
                        )

            if debug_taps:
                nc.sync.dma_start(at_dbg[:], attnT[:])

            # ---------------- Phase C: output projection ----------------
            with (
                tc.tile_pool(name="psumC", bufs=2, space="PSUM") as psumC,
                tc.tile_pool(name="outp", bufs=3) as outp,
            ):
                for m in range(ET):
                    ps = psumC.tile([P, LQ], F32, tag="fc")
                    for n in range(NQ):
                        for kf in range(2):
                            nc.tensor.matmul(
                                ps[:, n * 512:(n + 1) * 512],
                                fcw_s[:, kf, m * P:(m + 1) * P],
                                attnT[:, kf, n * 512:(n + 1) * 512],
                                start=(kf == 0),
                                stop=(kf == 1),
                            )
                    ob = outp.tile([P, LQ], F32, tag="ob")
                    nc.vector.tensor_copy(ob[:], ps[:])
                    nc.sync.dma_start(yT[m * P:(m + 1) * P, :], ob[:])

    nc.compile()
    return nc


_NC_CACHE: dict = {}


def _get_nc() -> bass.Bass:
    if "nc" not in _NC_CACHE:
        _NC_CACHE["nc"] = build_nc()
    return _NC_CACHE["nc"]


def make_in_maps(x, context, pad_mask, Wq, Wk, Wv, fc_w):
    x = np.asarray(x, dtype=np.float32)
    context = np.asarray(context, dtype=np.float32)
    pad_mask = np.asarray(pad_mask).astype(bool)
    Wq = np.asarray(Wq, dtype=np.float32)
    Wk = np.asarray(Wk, dtype=np.float32)
    Wv = np.asarray(Wv, dtype=np.float32)
    fc_w = np.asarray(fc_w, dtype=np.float32)

    xT = np.ascontiguousarray(x.transpose(0, 2, 1))                 # [B, E, LQ]
    cT = np.ascontiguousarray(context.transpose(0, 2, 1))           # [B, E, LKV]
    keepT = np.ascontiguousarray(
        (~pad_mask).transpose(0, 2, 1)
    ).astype(ml_dtypes.bfloat16)                                    # [B, LKV, LQ]

    in_maps = []
    for c in range(NCORES):
        b, hg = divmod(c, HGROUPS)
        fsl = slice(hg * FP, (hg + 1) * FP)
        in_maps.append(
            {
                "xT": xT[b],
                "ctxT": cT[b],
                "maskT": keepT[b],
                "wqT": np.ascontiguousarray(Wq[fsl, :].T),
                "wkT": np.ascontiguousarray(Wk[fsl, :].T),
                "wvT": np.ascontiguousarray(Wv[fsl, :].T),
                "fcwT": np.ascontiguousarray(fc_w[:, fsl].T).astype(ml_dtypes.bfloat16),
            }
        )
    return in_maps


def _combine(outs, fc_b):
    fc_b = np.asarray(fc_b, dtype=np.float32)
    y = np.empty((B, LQ, E), dtype=np.float32)
    for b in range(B):
        acc = outs[HGROUPS * b]
        for g in range(1, HGROUPS):
            acc = acc + outs[HGROUPS * b + g]
        y[b] = acc.T + fc_b
    return y


def run_traced(x, context, pad_mask, Wq, Wk, Wv, fc_w, fc_b, trace=False):
    nc = _get_nc()
    in_maps = make_in_maps(x, context, pad_mask, Wq, Wk, Wv, fc_w)
    res = run_bass_kernel_spmd(nc, in_maps, list(range(NCORES)), trace=trace)
    outs = [r["yT"] for r in res.results]
    return _combine(outs, fc_b), res


def kernel(x, context, pad_mask, Wq, Wk, Wv, fc_w, fc_b):
    y, _ = run_traced(x, context, pad_mask, Wq, Wk, Wv, fc_w, fc_b, trace=False)
    return y
